# revision 30
# baseline (speedup 1.0000x reference)
"""Trainium2 Bass kernel for top-2 MoE (nn_ExpertMemory).

Model (reference semantics):
    logits = x @ gate_w + gate_b          # (N, E)
    probs  = softmax(logits)
    gates, idx = top_k(probs, 2)
    out[n] = sum_k gates[n,k] * (relu(x[n] @ w1[e] + b1[e]) @ w2[e] + b2[e]),
             e = idx[n,k]
(The reference runs every expert densely, but combine weights are zero off
the top-2, so routed computation is mathematically identical.)

Strategy: data-parallel over tokens across 8 NeuronCores (1024 tokens each).
Each core, fully on device:
  1. gate matmul (true fp32) + softmax + top-2 on its tokens; per-(token,
     rank) gate values are also written to a small DRAM table ginter[2t+r+1]
  2. per-expert token lists via sparse_gather over candidate encodings;
     the junk tail beyond the found count (HW leaves arbitrary values) is
     neutralized with an on-device count mask
  3. dispatch via dma_gather(transpose=True) of token rows from DRAM
     directly into C-major SBUF layout; slot-ordered gate values are
     fetched with a second (tiny) dma_gather from ginter
  4. layer 1 slot-moving (out [h, slots], b1 as activation bias); layer 2
     with h as the stationary operand so the output lands token-major
     [slots, C] in PSUM; the gate is applied as the per-partition `scale`
     of the PSUM->SBUF copy
  5. combine via dma_scatter_add of bf16 rows into yT, which is
     pre-initialized with the gate-weighted b2 correction
     (sum_r g_r*b2[e_r]) computed by a tiny matmul. Pad slots scatter to a
     trash row: CCE read-modify-write is not atomic, so they must never
     alias real rows of the same scatter.
All tile pools live outside the repeat loop and the routing staging buffers
(cbuf/ginter) are double-buffered by repeat parity, so consecutive
iterations pipeline (stage A of rep i+1 overlaps stage B of rep i).
Slot capacities are specialized per run from a host-side replica of the
routing (inputs only, margin 16); the device computes everything itself.
"""

import numpy as np
from contextlib import ExitStack

import concourse.bass as bass
import concourse.tile as tile
import concourse.mybir as mybir
from concourse import bacc

dt = mybir.dt
AF = mybir.ActivationFunctionType
ALU = mybir.AluOpType
AX = mybir.AxisListType

P = 128

# problem constants
B, T, C, E, H, TOPK = 4, 2048, 1024, 8, 2048, 2
NCORE = 8
NTOK = B * T // NCORE  # tokens per core
TCH = NTOK // P        # token chunks (8)
CK = C // P            # C chunks (8)
HK = H // P            # H chunks (16)
GW64 = 64              # ginter row width (64 f32 = 256 B, gather minimum)
NH = NTOK // 512       # 512-wide token halves for the gate matmul


def _tiles(s):
    """Split slot range s into moving tiles: full 512s, then the remainder
    (16-aligned). Tile starts are 128-aligned so L2 sub-tiles line up with
    the global slot chunks."""
    out = []
    off = 0
    rem = s
    while rem > 512:
        out.append((off, 512))
        off += 512
        rem -= 512
    if rem:
        out.append((off, rem))
    return out


def build_program(S, ntok=NTOK, level=9, repeat=1):
    nc = _build(S, ntok=ntok, level=level, repeat=repeat)
    nc.compile()
    return nc


def _build(S, ntok=NTOK, level=9, repeat=1):
    """S: per-expert slot capacities (multiples of 16, each <= 512)."""
    S = [int(s) for s in S]
    assert all(s % 16 == 0 and 16 <= s <= 512 for s in S)
    S128 = [(s + 127) // 128 * 128 for s in S]

    nc = bacc.Bacc("TRN2", target_bir_lowering=False, debug=False,
                   num_swdge_queues=2)

    f32, bf16 = dt.float32, dt.bfloat16
    xT = nc.dram_tensor("xT", [C, ntok], f32, kind="ExternalInput").ap()
    xtm = nc.dram_tensor("xtm", [ntok, C], bf16, kind="ExternalInput").ap()
    gw = nc.dram_tensor("gw", [C, E], f32, kind="ExternalInput").ap()
    gb = nc.dram_tensor("gb", [E, 1], f32, kind="ExternalInput").ap()
    w1 = nc.dram_tensor("w1", [E, HK, P, CK * P], bf16,
                        kind="ExternalInput").ap()
    b1 = nc.dram_tensor("b1", [E, H, 1], f32, kind="ExternalInput").ap()
    w2 = nc.dram_tensor("w2", [E, H, C], bf16, kind="ExternalInput").ap()
    b2e = nc.dram_tensor("b2e", [E, C], f32, kind="ExternalInput").ap()
    id8 = nc.dram_tensor("id8", [E, E], f32, kind="ExternalInput").ap()
    id128 = nc.dram_tensor("id128", [P, P], f32, kind="ExternalInput").ap()
    # +16 rows: trash target for pad-slot scatter writes (their payload is
    # zero, but pointing them at real rows would race the real adds within
    # the same scatter DMA — CCE read-modify-write is not atomic)
    yT = nc.dram_tensor("yT", [ntok + 16, C], bf16, kind="ExternalOutput").ap()

    # routing staging, double-buffered by repeat parity
    cbufG = nc.dram_tensor("cbufG", [2, E, ntok], f32).ap()  # 2t+r+1 | -1
    cbufT = nc.dram_tensor("cbufT", [2, E, ntok], f32).ap()  # t+1 | -1
    # per-(token, rank) gate values, row 1+2t+r; row 0 zeroed for pads
    ginter = nc.dram_tensor("ginter", [2, 2 * ntok + 16, GW64], f32).ap()

    with tile.TileContext(nc) as tc, ExitStack() as ctx:
        cpool = ctx.enter_context(tc.tile_pool(name="const", bufs=1))
        gpool = ctx.enter_context(tc.tile_pool(name="gk", bufs=2))
        sa = ctx.enter_context(tc.tile_pool(name="sa", bufs=2))
        sa1 = ctx.enter_context(tc.tile_pool(name="sa1", bufs=2))
        xtp = ctx.enter_context(tc.tile_pool(name="xt", bufs=3))
        mb = ctx.enter_context(tc.tile_pool(name="mb", bufs=3))
        w1p = ctx.enter_context(tc.tile_pool(name="w1p", bufs=6))
        w2p = ctx.enter_context(tc.tile_pool(name="w2p", bufs=2))
        xgp = ctx.enter_context(tc.tile_pool(name="xgp", bufs=3))
        hp = ctx.enter_context(tc.tile_pool(name="hp", bufs=2))
        ystp = ctx.enter_context(tc.tile_pool(name="ystp", bufs=2))
        ycp = ctx.enter_context(tc.tile_pool(name="ycp", bufs=1))
        pgp = ctx.enter_context(tc.tile_pool(name="pgp", bufs=1,
                                             space="PSUM"))
        pms = ctx.enter_context(tc.tile_pool(name="pms", bufs=1,
                                             space="PSUM"))
        p1 = ctx.enter_context(tc.tile_pool(name="p1", bufs=2, space="PSUM"))
        p2 = ctx.enter_context(tc.tile_pool(name="p2", bufs=2, space="PSUM"))

        # ---- constants (loaded once) ----
        gwsb = cpool.tile([P, CK * E], f32)
        nc.sync.dma_start(gwsb[:].rearrange("p (k e) -> p k e", e=E),
                          gw.rearrange("(k p) e -> p k e", p=P))
        id8sb = cpool.tile([E, E], f32)
        nc.sync.dma_start(id8sb[:], id8)
        id128sb = cpool.tile([P, P], f32)
        nc.sync.dma_start(id128sb[:], id128)
        gbsb = cpool.tile([E, 1], f32)
        nc.sync.dma_start(gbsb[:], gb)
        b2sb = cpool.tile([E, C], f32)
        nc.sync.dma_start(b2sb[:], b2e)
        iotaE_i = cpool.tile([P, TCH * E], dt.int32)
        nc.gpsimd.iota(iotaE_i[:], pattern=[[0, TCH], [1, E]], base=0,
                       channel_multiplier=0)
        iotaE = cpool.tile([P, TCH * E], f32)
        nc.vector.tensor_copy(iotaE[:], iotaE_i[:])
        toks_i = cpool.tile([P, TCH], dt.int32)
        nc.gpsimd.iota(toks_i[:], pattern=[[P, TCH]], base=0,
                       channel_multiplier=1)
        toksf = cpool.tile([P, TCH], f32)
        nc.vector.tensor_copy(toksf[:], toks_i[:])
        slotio_i = cpool.tile([16, 512 // 16], dt.int32)
        nc.gpsimd.iota(slotio_i[:], pattern=[[16, 512 // 16]], base=0,
                       channel_multiplier=1)
        slotio = cpool.tile([16, 512 // 16], f32)
        nc.vector.tensor_copy(slotio[:], slotio_i[:])
        ones16 = cpool.tile([P, 16], f32)
        nc.vector.memset(ones16[:], 1.0)

        for rep in range(repeat):
            par = rep % 2
            cbG = cbufG[par]
            cbT = cbufT[par]
            gint = ginter[par]

            # =============== Stage A: gate + routing ===============
            # gate logits, expert-major: lgT[e, tok]. True fp32 matmul:
            # fp32r is reduced-precision on HW and would flip top-2 picks.
            lgT = sa1.tile([E, ntok], f32, tag="lgT")
            for nh in range(NH):
                lgps = pgp.tile([E, 512], f32, space="PSUM", tag="lgp")
                for k in range(CK):
                    xt = xtp.tile([P, 512], f32, tag="xt")
                    nc.sync.dma_start(
                        xt[:], xT[k * P:(k + 1) * P,
                                  nh * 512:(nh + 1) * 512])
                    nc.tensor.matmul(lgps[:],
                                     lhsT=gwsb[:, k * E:(k + 1) * E],
                                     rhs=xt[:],
                                     start=(k == 0), stop=(k == CK - 1))
                nc.vector.tensor_scalar_add(lgT[:, nh * 512:(nh + 1) * 512],
                                            lgps[:], gbsb[:, :1])
            # transpose to token-major [128, TCH, e]
            lg = sa1.tile([P, TCH, E], f32, tag="lg")
            for t in range(TCH):
                ps = pms.tile([P, E], f32, space="PSUM", tag="misc")
                nc.tensor.transpose(ps[:], lgT[:, t * P:(t + 1) * P],
                                    id8sb[:])
                nc.scalar.activation(lg[:, t, :], ps[:], AF.Copy)
            # softmax over experts
            mx = sa.tile([P, TCH], f32, tag="mx")
            nc.vector.tensor_reduce(mx[:], lg[:], axis=AX.X, op=ALU.max)
            xm = sa.tile([P, TCH, E], f32, tag="xm")
            nc.vector.tensor_tensor(out=xm[:], in0=lg[:],
                                    in1=mx[:].to_broadcast([P, TCH, E]),
                                    op=ALU.subtract)
            ex = sa.tile([P, TCH, E], f32, tag="ex")
            nc.scalar.activation(ex[:], xm[:], AF.Exp)
            sm = sa.tile([P, TCH], f32, tag="sm")
            nc.vector.tensor_reduce(sm[:], ex[:], axis=AX.X, op=ALU.add)
            rs = sa.tile([P, TCH], f32, tag="rs")
            nc.vector.reciprocal(rs[:], sm[:])
            probs = sa.tile([P, TCH, E], f32, tag="probs")
            nc.vector.tensor_tensor(out=probs[:], in0=ex[:],
                                    in1=rs[:].to_broadcast([P, TCH, E]),
                                    op=ALU.mult)
            # top-2 by logits (same order as by probs)
            mig = sa.tile([P, TCH, 8], dt.uint32, tag="mig")
            for t in range(TCH):
                mv = sa.tile([P, 8], f32, tag="mv")
                nc.vector.max(mv[:], lg[:, t, :])
                nc.vector.max_index(mig[:, t, :], mv[:], lg[:, t, :])
            migf = sa.tile([P, TCH, 8], f32, tag="migf")
            nc.vector.tensor_copy(migf[:], mig[:])

            A = []  # one-hot masks per rank [P, TCH, e]
            g = []
            for r in range(2):
                Ar = sa1.tile([P, TCH, E], f32, tag=f"A{r}")
                nc.vector.tensor_tensor(
                    out=Ar[:],
                    in0=migf[:, :, r:r + 1].to_broadcast([P, TCH, E]),
                    in1=iotaE[:].rearrange("p (t e) -> p t e", e=E),
                    op=ALU.is_equal)
                gr = gpool.tile([P, TCH], f32, tag=f"g{r}")
                tmp = sa.tile([P, TCH, E], f32, tag="gt")
                nc.vector.tensor_tensor(out=tmp[:], in0=probs[:], in1=Ar[:],
                                        op=ALU.mult)
                nc.vector.tensor_reduce(gr[:], tmp[:], axis=AX.X, op=ALU.add)
                A.append(Ar)
                g.append(gr)
            M = sa1.tile([P, TCH, E], f32, tag="M")
            nc.vector.tensor_tensor(out=M[:], in0=A[0][:], in1=A[1][:],
                                    op=ALU.add)

            # per-expert token counts, replicated on 16 partitions (used to
            # mask off sparse_gather's junk tail beyond the found count)
            Mre = sa.tile([P, E, TCH], f32, tag="Mre")
            nc.vector.tensor_copy(Mre[:], M[:].rearrange("p t e -> p e t"))
            cntp = pms.tile([16, E * TCH], f32, space="PSUM", tag="misc")
            nc.tensor.matmul(cntp[:], lhsT=ones16[:],
                             rhs=Mre[:].rearrange("p e t -> p (e t)"),
                             start=True, stop=True)
            cntet = sa.tile([16, E, TCH], f32, tag="cntet")
            nc.scalar.activation(cntet[:],
                                 cntp[:].rearrange("p (e t) -> p e t", e=E),
                                 AF.Copy)
            cnt16 = gpool.tile([16, E], f32, tag="cnt16")
            nc.vector.tensor_reduce(cnt16[:], cntet[:], axis=AX.X, op=ALU.add)

            if level < 1:
                continue
            # candidate encodings (+1-shifted so sparse-gather pads, which
            # are <= 0, can be clamped to the zero row / token 0):
            #   G = 2*tok + r + 1 (else -1), T = tok + 1 (else -1)
            tokp2 = sa.tile([P, TCH], f32, tag="tokp2")
            nc.vector.tensor_scalar_add(tokp2[:], toksf[:], 2.0)
            tok2 = sa.tile([P, TCH], f32, tag="tok2")
            nc.vector.tensor_scalar(tok2[:], toksf[:], 2.0, 2.0,
                                    op0=ALU.mult, op1=ALU.add)
            candG = sa1.tile([P, TCH, E], f32, tag="candG")
            nc.vector.tensor_tensor(
                out=candG[:], in0=tok2[:].to_broadcast([P, TCH, E]),
                in1=M[:], op=ALU.mult)
            nc.vector.tensor_tensor(out=candG[:], in0=candG[:], in1=A[1][:],
                                    op=ALU.add)
            nc.vector.tensor_scalar_add(candG[:], candG[:], -1.0)
            candT = sa1.tile([P, TCH, E], f32, tag="candT")
            nc.vector.tensor_tensor(
                out=candT[:], in0=tokp2[:].to_broadcast([P, TCH, E]),
                in1=M[:], op=ALU.mult)
            nc.vector.tensor_scalar_add(candT[:], candT[:], -1.0)
            for ei in range(E):
                nc.scalar.dma_start(
                    cbG[ei, :].rearrange("(t p) -> p t", p=P),
                    candG[:, :, ei])
                nc.scalar.dma_start(
                    cbT[ei, :].rearrange("(t p) -> p t", p=P),
                    candT[:, :, ei])

            # ---- ginter: per-(token, rank) gates, rows 1 + 2t + r ----
            zrow = sa.tile([1, GW64], f32, tag="zrow")
            nc.vector.memset(zrow[:], 0.0)
            nc.scalar.dma_start(gint[0:1, :], zrow[:])
            for r in range(2):
                g64 = sa.tile([P, TCH, GW64], f32, tag=f"g64_{r}")
                nc.vector.tensor_scalar_add(
                    g64[:], g[r][:].to_broadcast([P, TCH, GW64]), 0.0)
                nc.scalar.dma_start(
                    gint[1:1 + 2 * ntok, :].rearrange(
                        "(tch p two) f -> p tch two f",
                        p=P, two=2)[:, :, r, :],
                    g64[:])

            # ---- yT init: sum_r g_r * b2[e_r] ----
            wtok = sa1.tile([P, TCH, E], f32, tag="wtok")
            nc.vector.tensor_tensor(
                out=wtok[:], in0=A[0][:],
                in1=g[0][:].to_broadcast([P, TCH, E]), op=ALU.mult)
            wtk1 = sa.tile([P, TCH, E], f32, tag="wtk1")
            nc.vector.tensor_tensor(
                out=wtk1[:], in0=A[1][:],
                in1=g[1][:].to_broadcast([P, TCH, E]), op=ALU.mult)
            nc.vector.tensor_tensor(out=wtok[:], in0=wtok[:], in1=wtk1[:],
                                    op=ALU.add)
            wTe = sa1.tile([E, TCH * P], f32, tag="wTe")
            for t in range(TCH):
                pw = pms.tile([E, P], f32, space="PSUM", tag="misc")
                nc.tensor.transpose(pw[:], wtok[:, t, :], id128sb[:])
                nc.scalar.activation(wTe[:, t * P:(t + 1) * P], pw[:],
                                     AF.Copy)
            ycorr = ycp.tile([P, TCH, C], bf16, tag="ycorr")
            for t in range(TCH):
                for hh in range(2):
                    pc = pms.tile([P, 512], f32, space="PSUM", tag="misc")
                    nc.tensor.matmul(pc[:], lhsT=wTe[:, t * P:(t + 1) * P],
                                     rhs=b2sb[:, hh * 512:(hh + 1) * 512],
                                     start=True, stop=True)
                    nc.scalar.activation(ycorr[:, t, hh * 512:(hh + 1) * 512],
                                         pc[:], AF.Copy)
            nc.sync.dma_start(
                yT[0:ntok, :].rearrange("(tch p) c -> p tch c", p=P),
                ycorr[:])

            if level < 2:
                continue
            # =============== Stage B: expert MLP + scatter ===============
            for ei in range(E):
                Se = S[ei]
                Sg = S128[ei]
                nsub = (Se + 127) // 128
                w16 = Sg // 16
                # ---- token lists ----
                cwG = mb.tile([16, ntok // 16], f32, tag="cwG")
                nc.scalar.dma_start(
                    cwG[:], cbG[ei, :].rearrange("(f p) -> p f", p=16))
                cwT = mb.tile([16, ntok // 16], f32, tag="cwT")
                nc.scalar.dma_start(
                    cwT[:], cbT[ei, :].rearrange("(f p) -> p f", p=16))

                # junk-tail mask: slots >= count are diverted/neutralized
                msk = mb.tile([16, w16], f32, tag="msk")
                nc.vector.tensor_tensor(
                    out=msk[:], in0=slotio[:, :w16],
                    in1=cnt16[:, ei:ei + 1].to_broadcast([16, w16]),
                    op=ALU.is_lt)

                tkT = mb.tile([16, w16], f32, tag="tkT")
                nc.vector.memset(tkT[:], -1.0)
                nfdT = mb.tile([1, 1], dt.uint32, tag="nfdT")
                nc.gpsimd.sparse_gather(tkT[:, :Se // 16], cwT[:],
                                        num_found=nfdT[:])
                tkG = mb.tile([16, w16], f32, tag="tkG")
                nc.vector.memset(tkG[:], -1.0)
                nfdG = mb.tile([1, 1], dt.uint32, tag="nfdG")
                nc.gpsimd.sparse_gather(tkG[:, :Se // 16], cwG[:],
                                        num_found=nfdG[:])

                # x-row list: token ids, pads/junk clamped into range
                xff = mb.tile([16, w16], f32, tag="xff")
                nc.vector.tensor_scalar(xff[:], tkT[:], 1.0, float(ntok),
                                        op0=ALU.max, op1=ALU.min)
                nc.vector.tensor_scalar_add(xff[:], xff[:], -1.0)
                # scatter list: same tokens, junk tail -> trash row
                stf = mb.tile([16, Se // 16], f32, tag="stf")
                nc.vector.scalar_tensor_tensor(
                    out=stf[:], in0=xff[:, :Se // 16], scalar=float(-ntok),
                    in1=msk[:, :Se // 16], op0=ALU.add, op1=ALU.mult)
                nc.vector.tensor_scalar_add(stf[:], stf[:], float(ntok))
                # gate list: rows 1+2t+r of ginter, pads -> zero row
                gvf = mb.tile([16, w16], f32, tag="gvf")
                nc.vector.tensor_scalar(gvf[:], tkG[:], 0.0, float(2 * ntok),
                                        op0=ALU.max, op1=ALU.min)

                # single replicated index tile: [X | T | G]
                trip = mb.tile([P, 3 * w16], dt.int16, tag="trip")
                nc.vector.tensor_copy(trip[0:16, 0:w16], xff[:])
                nc.vector.tensor_copy(trip[0:16, w16:w16 + Se // 16], stf[:])
                nc.vector.tensor_copy(trip[0:16, 2 * w16:3 * w16], gvf[:])
                for sz in (16, 32, 64):
                    nc.scalar.dma_start(trip[sz:2 * sz, :], trip[0:sz, :])
                t16X = trip[:, 0:w16]
                t16T = trip[:, w16:w16 + Se // 16]
                t16G = trip[:, 2 * w16:3 * w16]

                # ---- dispatch gathers ----
                xg = xgp.tile([P, CK, Sg], bf16, tag="xg")
                nc.gpsimd.dma_gather(xg[:], xtm, t16X, Sg, Sg, C,
                                     transpose=True)
                ggt = xgp.tile([P, Sg // 128, GW64], f32, tag="gg")
                nc.gpsimd.dma_gather(ggt[:], gint, t16G, Sg, Sg, GW64)

                b1e = mb.tile([P, HK], f32, tag="b1e")
                nc.scalar.dma_start(
                    b1e[:].rearrange("p (k o) -> p k o", o=1),
                    b1[ei].rearrange("(k p) one -> p k one", p=P))

                # ---- w2 resident for this expert ----
                w2t = []
                for hk in range(HK):
                    wt = w2p.tile([P, C], bf16, tag=f"w2_{hk}")
                    nc.sync.dma_start(
                        wt[:], w2[ei, hk * P:(hk + 1) * P, :])
                    w2t.append(wt)

                yst = ystp.tile([P, nsub, C], bf16, tag="yst")
                if level < 3:
                    continue
                for (woff, W) in _tiles(Se):
                    # layer 1: out [h, slots]
                    hs = []
                    for hk in range(HK):
                        wrow = w1p.tile([P, CK * P], bf16, tag="w1r")
                        nc.sync.dma_start(wrow[:], w1[ei, hk])
                        ps = p1.tile([P, W], f32, space="PSUM", tag="ps1")
                        for k in range(CK):
                            nc.tensor.matmul(
                                ps[:], lhsT=wrow[:, k * P:(k + 1) * P],
                                rhs=xg[:, k, woff:woff + W],
                                start=(k == 0), stop=(k == CK - 1))
                        ht = hp.tile([P, W], bf16, tag=f"h{hk}")
                        nc.scalar.activation(ht[:], ps[:], AF.Relu,
                                             bias=b1e[:, hk:hk + 1])
                        hs.append(ht)
                    if level < 4:
                        continue
                    # layer 2: h stationary -> out token-major [slots, C]
                    for sub in range((W + 127) // 128):
                        lo = sub * 128
                        wsub = min(128, W - lo)
                        gsub = (woff + lo) // 128
                        psA = p2.tile([P, 512], f32, space="PSUM", tag="ps2a")
                        psB = p2.tile([P, 512], f32, space="PSUM", tag="ps2b")
                        for hk in range(HK):
                            nc.tensor.matmul(
                                psA[0:wsub, :],
                                lhsT=hs[hk][:, lo:lo + wsub],
                                rhs=w2t[hk][:, 0:512],
                                start=(hk == 0), stop=(hk == HK - 1))
                            nc.tensor.matmul(
                                psB[0:wsub, :],
                                lhsT=hs[hk][:, lo:lo + wsub],
                                rhs=w2t[hk][:, 512:1024],
                                start=(hk == 0), stop=(hk == HK - 1))
                        nc.scalar.activation(yst[0:wsub, gsub, 0:512],
                                             psA[0:wsub, :], AF.Copy,
                                             scale=ggt[0:wsub, gsub, 0:1])
                        nc.scalar.activation(yst[0:wsub, gsub, 512:1024],
                                             psB[0:wsub, :], AF.Copy,
                                             scale=ggt[0:wsub, gsub, 0:1])
                if level < 5:
                    continue
                # ---- combine: scatter-add token rows into yT ----
                nc.gpsimd.dma_scatter_add(yT, yst[:], t16T, Se, Se, C,
                                          queue_num=1)

    return nc


# ---------------- host side ----------------

def _host_caps(xf, gate_w, gate_b, ntok=NTOK, margin=16):
    """Slot capacities per expert from a host replica of the routing."""
    logits = xf.astype(np.float32) @ gate_w.astype(np.float32) + gate_b
    order = np.argpartition(-logits, TOPK - 1, axis=1)[:, :TOPK]
    ncore = xf.shape[0] // ntok
    counts = np.zeros((ncore, E), np.int64)
    for cc in range(ncore):
        sl = order[cc * ntok:(cc + 1) * ntok]
        counts[cc] = np.bincount(sl.ravel(), minlength=E)
    maxc = counts.max(axis=0)
    S = ((maxc + margin + 15) // 16) * 16
    assert S.max() <= 512, f"capacity overflow: {S}"
    return S.astype(np.int64)


def kernel(x, gate_w, gate_b, w1, b1, w2, b2):
    from concourse.bass_utils import run_bass_kernel_spmd
    import ml_dtypes

    x = np.asarray(x, np.float32)
    gate_w = np.asarray(gate_w, np.float32)
    gate_b = np.asarray(gate_b, np.float32)
    w1 = np.asarray(w1, np.float32)
    b1 = np.asarray(b1, np.float32)
    w2 = np.asarray(w2, np.float32)
    b2 = np.asarray(b2, np.float32)

    # w1 in lhsT-chunk layout: [E, HK, P(c in chunk), CK*P(h)]
    w1r = np.ascontiguousarray(
        (w1.reshape(E, CK, P, HK, P).transpose(0, 3, 2, 1, 4)
         .reshape(E, HK, P, C)).astype(ml_dtypes.bfloat16))
    w2b = np.ascontiguousarray(w2.astype(ml_dtypes.bfloat16))

    b, t, c = x.shape
    xf = x.reshape(b * t, c)
    S = _host_caps(xf, gate_w, gate_b)
    nc = build_program(S)

    shared = {
        "gw": gate_w,
        "gb": gate_b.reshape(E, 1).copy(),
        "w1": w1r,
        "b1": b1.reshape(E, H, 1).copy(),
        "w2": w2b,
        "b2e": b2,
        "id8": np.eye(E, dtype=np.float32),
        "id128": np.eye(P, dtype=np.float32),
    }
    in_maps = []
    for cc in range(NCORE):
        sl = xf[cc * NTOK:(cc + 1) * NTOK]
        m = dict(shared)
        m["xT"] = np.ascontiguousarray(sl.T)
        m["xtm"] = np.ascontiguousarray(sl.astype(ml_dtypes.bfloat16))
        in_maps.append(m)

    global LAST_BUILD, LAST_S
    LAST_BUILD = (nc, in_maps)
    LAST_S = S
    res = run_bass_kernel_spmd(nc, in_maps, core_ids=list(range(NCORE)))
    outs = [np.asarray(r["yT"][:NTOK]).astype(np.float32)
            for r in res.results]
    y = np.concatenate(outs, axis=0).reshape(b, t, c)
    return y


# revision 31
# speedup vs baseline: 1.0287x; 1.0287x over previous
"""Trainium2 Bass kernel for top-2 MoE (nn_ExpertMemory).

Model (reference semantics):
    logits = x @ gate_w + gate_b          # (N, E)
    probs  = softmax(logits)
    gates, idx = top_k(probs, 2)
    out[n] = sum_k gates[n,k] * (relu(x[n] @ w1[e] + b1[e]) @ w2[e] + b2[e]),
             e = idx[n,k]
(The reference runs every expert densely, but combine weights are zero off
the top-2, so routed computation is mathematically identical.)

Strategy: data-parallel over tokens across 8 NeuronCores (1024 tokens each).
Each core, fully on device:
  1. gate matmul (true fp32) + softmax + top-2 on its tokens; per-(token,
     rank) gate values are also written to a small DRAM table ginter[2t+r+1]
  2. per-expert token lists via sparse_gather over candidate encodings;
     the junk tail beyond the found count (HW leaves arbitrary values) is
     neutralized with an on-device count mask
  3. dispatch via dma_gather(transpose=True) of token rows from DRAM
     directly into C-major SBUF layout; slot-ordered gate values are
     fetched with a second (tiny) dma_gather from ginter
  4. layer 1 slot-moving (out [h, slots], b1 as activation bias); layer 2
     with h as the stationary operand so the output lands token-major
     [slots, C] in PSUM; the gate is applied as the per-partition `scale`
     of the PSUM->SBUF copy
  5. combine via dma_scatter_add of bf16 rows into yT, which is
     pre-initialized with the gate-weighted b2 correction
     (sum_r g_r*b2[e_r]) computed by a tiny matmul. Pad slots scatter to a
     trash row: CCE read-modify-write is not atomic, so they must never
     alias real rows of the same scatter.
All tile pools live outside the repeat loop and the routing staging buffers
(cbuf/ginter) are double-buffered by repeat parity, so consecutive
iterations pipeline (stage A of rep i+1 overlaps stage B of rep i).
Slot capacities are specialized per run from a host-side replica of the
routing (inputs only, margin 16); the device computes everything itself.
"""

import numpy as np
from contextlib import ExitStack

import concourse.bass as bass
import concourse.tile as tile
import concourse.mybir as mybir
from concourse import bacc

dt = mybir.dt
AF = mybir.ActivationFunctionType
ALU = mybir.AluOpType
AX = mybir.AxisListType

P = 128

# problem constants
B, T, C, E, H, TOPK = 4, 2048, 1024, 8, 2048, 2
NCORE = 8
NTOK = B * T // NCORE  # tokens per core
TCH = NTOK // P        # token chunks (8)
CK = C // P            # C chunks (8)
HK = H // P            # H chunks (16)
GW64 = 64              # ginter row width (64 f32 = 256 B, gather minimum)
NH = NTOK // 512       # 512-wide token halves for the gate matmul


def _tiles(s):
    """Split slot range s into moving tiles: full 512s, then the remainder
    (16-aligned). Tile starts are 128-aligned so L2 sub-tiles line up with
    the global slot chunks."""
    out = []
    off = 0
    rem = s
    while rem > 512:
        out.append((off, 512))
        off += 512
        rem -= 512
    if rem:
        out.append((off, rem))
    return out


def build_program(S, ntok=NTOK, level=9, repeat=1):
    nc = _build(S, ntok=ntok, level=level, repeat=repeat)
    nc.compile()
    return nc


def _build(S, ntok=NTOK, level=9, repeat=1):
    """S: per-expert slot capacities (multiples of 16, each <= 512)."""
    S = [int(s) for s in S]
    assert all(s % 16 == 0 and 16 <= s <= 512 for s in S)
    S128 = [(s + 127) // 128 * 128 for s in S]

    nc = bacc.Bacc("TRN2", target_bir_lowering=False, debug=False,
                   num_swdge_queues=2)

    f32, bf16 = dt.float32, dt.bfloat16
    xT = nc.dram_tensor("xT", [C, ntok], f32, kind="ExternalInput").ap()
    xtm = nc.dram_tensor("xtm", [ntok, C], bf16, kind="ExternalInput").ap()
    gw = nc.dram_tensor("gw", [C, E], f32, kind="ExternalInput").ap()
    gb = nc.dram_tensor("gb", [E, 1], f32, kind="ExternalInput").ap()
    w1 = nc.dram_tensor("w1", [E, HK, P, CK * P], bf16,
                        kind="ExternalInput").ap()
    b1 = nc.dram_tensor("b1", [E, H, 1], f32, kind="ExternalInput").ap()
    w2 = nc.dram_tensor("w2", [E, H, C], bf16, kind="ExternalInput").ap()
    b2e = nc.dram_tensor("b2e", [E, C], f32, kind="ExternalInput").ap()
    id8 = nc.dram_tensor("id8", [E, E], f32, kind="ExternalInput").ap()
    id128 = nc.dram_tensor("id128", [P, P], f32, kind="ExternalInput").ap()
    # +16 rows: trash target for pad-slot scatter writes (their payload is
    # zero, but pointing them at real rows would race the real adds within
    # the same scatter DMA — CCE read-modify-write is not atomic)
    yT = nc.dram_tensor("yT", [ntok + 16, C], bf16, kind="ExternalOutput").ap()

    # routing staging, double-buffered by repeat parity
    cbufG = nc.dram_tensor("cbufG", [2, E, ntok], f32).ap()  # 2t+r+1 | -1
    cbufT = nc.dram_tensor("cbufT", [2, E, ntok], f32).ap()  # t+1 | -1
    # per-(token, rank) gate values, row 1+2t+r; row 0 zeroed for pads
    ginter = nc.dram_tensor("ginter", [2, 2 * ntok + 16, GW64], f32).ap()

    with tile.TileContext(nc) as tc, ExitStack() as ctx:
        cpool = ctx.enter_context(tc.tile_pool(name="const", bufs=1))
        gpool = ctx.enter_context(tc.tile_pool(name="gk", bufs=2))
        sa = ctx.enter_context(tc.tile_pool(name="sa", bufs=2))
        sa1 = ctx.enter_context(tc.tile_pool(name="sa1", bufs=2))
        xtp = ctx.enter_context(tc.tile_pool(name="xt", bufs=3))
        mb = ctx.enter_context(tc.tile_pool(name="mb", bufs=3))
        w1p = ctx.enter_context(tc.tile_pool(name="w1p", bufs=6))
        w2p = ctx.enter_context(tc.tile_pool(name="w2p", bufs=2))
        xgp = ctx.enter_context(tc.tile_pool(name="xgp", bufs=3))
        hp = ctx.enter_context(tc.tile_pool(name="hp", bufs=2))
        ystp = ctx.enter_context(tc.tile_pool(name="ystp", bufs=2))
        ycp = ctx.enter_context(tc.tile_pool(name="ycp", bufs=1))
        pgp = ctx.enter_context(tc.tile_pool(name="pgp", bufs=1,
                                             space="PSUM"))
        pms = ctx.enter_context(tc.tile_pool(name="pms", bufs=1,
                                             space="PSUM"))
        p1 = ctx.enter_context(tc.tile_pool(name="p1", bufs=2, space="PSUM"))
        p2 = ctx.enter_context(tc.tile_pool(name="p2", bufs=2, space="PSUM"))

        # ---- constants (loaded once) ----
        gwsb = cpool.tile([P, CK * E], f32)
        nc.sync.dma_start(gwsb[:].rearrange("p (k e) -> p k e", e=E),
                          gw.rearrange("(k p) e -> p k e", p=P))
        id8sb = cpool.tile([E, E], f32)
        nc.sync.dma_start(id8sb[:], id8)
        id128sb = cpool.tile([P, P], f32)
        nc.sync.dma_start(id128sb[:], id128)
        gbsb = cpool.tile([E, 1], f32)
        nc.sync.dma_start(gbsb[:], gb)
        b2sb = cpool.tile([E, C], f32)
        nc.sync.dma_start(b2sb[:], b2e)
        iotaE_i = cpool.tile([P, TCH * E], dt.int32)
        nc.gpsimd.iota(iotaE_i[:], pattern=[[0, TCH], [1, E]], base=0,
                       channel_multiplier=0)
        iotaE = cpool.tile([P, TCH * E], f32)
        nc.vector.tensor_copy(iotaE[:], iotaE_i[:])
        toks_i = cpool.tile([P, TCH], dt.int32)
        nc.gpsimd.iota(toks_i[:], pattern=[[P, TCH]], base=0,
                       channel_multiplier=1)
        toksf = cpool.tile([P, TCH], f32)
        nc.vector.tensor_copy(toksf[:], toks_i[:])
        slotio_i = cpool.tile([16, 512 // 16], dt.int32)
        nc.gpsimd.iota(slotio_i[:], pattern=[[16, 512 // 16]], base=0,
                       channel_multiplier=1)
        slotio = cpool.tile([16, 512 // 16], f32)
        nc.vector.tensor_copy(slotio[:], slotio_i[:])
        ones16 = cpool.tile([P, 16], f32)
        nc.vector.memset(ones16[:], 1.0)

        def stage_a(rep):
            par = rep % 2
            cbG = cbufG[par]
            cbT = cbufT[par]
            gint = ginter[par]

            # =============== Stage A: gate + routing ===============
            # gate logits, expert-major: lgT[e, tok]. True fp32 matmul:
            # fp32r is reduced-precision on HW and would flip top-2 picks.
            lgT = sa1.tile([E, ntok], f32, tag="lgT")
            for nh in range(NH):
                lgps = pgp.tile([E, 512], f32, space="PSUM", tag="lgp")
                for k in range(CK):
                    xt = xtp.tile([P, 512], f32, tag="xt")
                    nc.sync.dma_start(
                        xt[:], xT[k * P:(k + 1) * P,
                                  nh * 512:(nh + 1) * 512])
                    nc.tensor.matmul(lgps[:],
                                     lhsT=gwsb[:, k * E:(k + 1) * E],
                                     rhs=xt[:],
                                     start=(k == 0), stop=(k == CK - 1))
                nc.vector.tensor_scalar_add(lgT[:, nh * 512:(nh + 1) * 512],
                                            lgps[:], gbsb[:, :1])
            # transpose to token-major [128, TCH, e]
            lg = sa1.tile([P, TCH, E], f32, tag="lg")
            for t in range(TCH):
                ps = pms.tile([P, E], f32, space="PSUM", tag="misc")
                nc.tensor.transpose(ps[:], lgT[:, t * P:(t + 1) * P],
                                    id8sb[:])
                nc.scalar.activation(lg[:, t, :], ps[:], AF.Copy)
            # softmax over experts
            mx = sa.tile([P, TCH], f32, tag="mx")
            nc.vector.tensor_reduce(mx[:], lg[:], axis=AX.X, op=ALU.max)
            xm = sa.tile([P, TCH, E], f32, tag="xm")
            nc.vector.tensor_tensor(out=xm[:], in0=lg[:],
                                    in1=mx[:].to_broadcast([P, TCH, E]),
                                    op=ALU.subtract)
            ex = sa.tile([P, TCH, E], f32, tag="ex")
            nc.scalar.activation(ex[:], xm[:], AF.Exp)
            sm = sa.tile([P, TCH], f32, tag="sm")
            nc.vector.tensor_reduce(sm[:], ex[:], axis=AX.X, op=ALU.add)
            rs = sa.tile([P, TCH], f32, tag="rs")
            nc.vector.reciprocal(rs[:], sm[:])
            probs = sa.tile([P, TCH, E], f32, tag="probs")
            nc.vector.tensor_tensor(out=probs[:], in0=ex[:],
                                    in1=rs[:].to_broadcast([P, TCH, E]),
                                    op=ALU.mult)
            # top-2 by logits (same order as by probs)
            mig = sa.tile([P, TCH, 8], dt.uint32, tag="mig")
            for t in range(TCH):
                mv = sa.tile([P, 8], f32, tag="mv")
                nc.vector.max(mv[:], lg[:, t, :])
                nc.vector.max_index(mig[:, t, :], mv[:], lg[:, t, :])
            migf = sa.tile([P, TCH, 8], f32, tag="migf")
            nc.vector.tensor_copy(migf[:], mig[:])

            A = []  # one-hot masks per rank [P, TCH, e]
            g = []
            for r in range(2):
                Ar = sa1.tile([P, TCH, E], f32, tag=f"A{r}")
                nc.vector.tensor_tensor(
                    out=Ar[:],
                    in0=migf[:, :, r:r + 1].to_broadcast([P, TCH, E]),
                    in1=iotaE[:].rearrange("p (t e) -> p t e", e=E),
                    op=ALU.is_equal)
                gr = gpool.tile([P, TCH], f32, tag=f"g{r}")
                tmp = sa.tile([P, TCH, E], f32, tag="gt")
                nc.vector.tensor_tensor(out=tmp[:], in0=probs[:], in1=Ar[:],
                                        op=ALU.mult)
                nc.vector.tensor_reduce(gr[:], tmp[:], axis=AX.X, op=ALU.add)
                A.append(Ar)
                g.append(gr)
            M = sa1.tile([P, TCH, E], f32, tag="M")
            nc.vector.tensor_tensor(out=M[:], in0=A[0][:], in1=A[1][:],
                                    op=ALU.add)

            # per-expert token counts, replicated on 16 partitions (used to
            # mask off sparse_gather's junk tail beyond the found count)
            Mre = sa.tile([P, E, TCH], f32, tag="Mre")
            nc.vector.tensor_copy(Mre[:], M[:].rearrange("p t e -> p e t"))
            cntp = pms.tile([16, E * TCH], f32, space="PSUM", tag="misc")
            nc.tensor.matmul(cntp[:], lhsT=ones16[:],
                             rhs=Mre[:].rearrange("p e t -> p (e t)"),
                             start=True, stop=True)
            cntet = sa.tile([16, E, TCH], f32, tag="cntet")
            nc.scalar.activation(cntet[:],
                                 cntp[:].rearrange("p (e t) -> p e t", e=E),
                                 AF.Copy)
            cnt16 = gpool.tile([16, E], f32, tag="cnt16")
            nc.vector.tensor_reduce(cnt16[:], cntet[:], axis=AX.X, op=ALU.add)

            if level < 1:
                return None
            # candidate encodings (+1-shifted so sparse-gather pads, which
            # are <= 0, can be clamped to the zero row / token 0):
            #   G = 2*tok + r + 1 (else -1), T = tok + 1 (else -1)
            tokp2 = sa.tile([P, TCH], f32, tag="tokp2")
            nc.vector.tensor_scalar_add(tokp2[:], toksf[:], 2.0)
            tok2 = sa.tile([P, TCH], f32, tag="tok2")
            nc.vector.tensor_scalar(tok2[:], toksf[:], 2.0, 2.0,
                                    op0=ALU.mult, op1=ALU.add)
            candG = sa1.tile([P, TCH, E], f32, tag="candG")
            nc.vector.tensor_tensor(
                out=candG[:], in0=tok2[:].to_broadcast([P, TCH, E]),
                in1=M[:], op=ALU.mult)
            nc.vector.tensor_tensor(out=candG[:], in0=candG[:], in1=A[1][:],
                                    op=ALU.add)
            nc.vector.tensor_scalar_add(candG[:], candG[:], -1.0)
            candT = sa1.tile([P, TCH, E], f32, tag="candT")
            nc.vector.tensor_tensor(
                out=candT[:], in0=tokp2[:].to_broadcast([P, TCH, E]),
                in1=M[:], op=ALU.mult)
            nc.vector.tensor_scalar_add(candT[:], candT[:], -1.0)
            for ei in range(E):
                nc.scalar.dma_start(
                    cbG[ei, :].rearrange("(t p) -> p t", p=P),
                    candG[:, :, ei])
                nc.scalar.dma_start(
                    cbT[ei, :].rearrange("(t p) -> p t", p=P),
                    candT[:, :, ei])

            # ---- ginter: per-(token, rank) gates, rows 1 + 2t + r ----
            zrow = sa.tile([1, GW64], f32, tag="zrow")
            nc.vector.memset(zrow[:], 0.0)
            nc.scalar.dma_start(gint[0:1, :], zrow[:])
            for r in range(2):
                g64 = sa.tile([P, TCH, GW64], f32, tag=f"g64_{r}")
                nc.vector.tensor_scalar_add(
                    g64[:], g[r][:].to_broadcast([P, TCH, GW64]), 0.0)
                nc.scalar.dma_start(
                    gint[1:1 + 2 * ntok, :].rearrange(
                        "(tch p two) f -> p tch two f",
                        p=P, two=2)[:, :, r, :],
                    g64[:])

            # ---- yT init: sum_r g_r * b2[e_r] ----
            wtok = sa1.tile([P, TCH, E], f32, tag="wtok")
            nc.vector.tensor_tensor(
                out=wtok[:], in0=A[0][:],
                in1=g[0][:].to_broadcast([P, TCH, E]), op=ALU.mult)
            wtk1 = sa.tile([P, TCH, E], f32, tag="wtk1")
            nc.vector.tensor_tensor(
                out=wtk1[:], in0=A[1][:],
                in1=g[1][:].to_broadcast([P, TCH, E]), op=ALU.mult)
            nc.vector.tensor_tensor(out=wtok[:], in0=wtok[:], in1=wtk1[:],
                                    op=ALU.add)
            wTe = sa1.tile([E, TCH * P], f32, tag="wTe")
            for t in range(TCH):
                pw = pms.tile([E, P], f32, space="PSUM", tag="misc")
                nc.tensor.transpose(pw[:], wtok[:, t, :], id128sb[:])
                nc.scalar.activation(wTe[:, t * P:(t + 1) * P], pw[:],
                                     AF.Copy)
            ycorr = ycp.tile([P, TCH, C], bf16, tag="ycorr")
            for t in range(TCH):
                for hh in range(2):
                    pc = pms.tile([P, 512], f32, space="PSUM", tag="misc")
                    nc.tensor.matmul(pc[:], lhsT=wTe[:, t * P:(t + 1) * P],
                                     rhs=b2sb[:, hh * 512:(hh + 1) * 512],
                                     start=True, stop=True)
                    nc.scalar.activation(ycorr[:, t, hh * 512:(hh + 1) * 512],
                                         pc[:], AF.Copy)
            return dict(cnt16=cnt16, ycorr=ycorr, cbG=cbG, cbT=cbT,
                        gint=gint)

        def stage_b(rep, actx):
            cnt16 = actx["cnt16"]
            cbG, cbT, gint = actx["cbG"], actx["cbT"], actx["gint"]
            # yT init (emitted here so the WAW chain with the previous
            # rep's scatter-adds stays in the right order)
            nc.sync.dma_start(
                yT[0:ntok, :].rearrange("(tch p) c -> p tch c", p=P),
                actx["ycorr"][:])
            for ei in range(E):
                Se = S[ei]
                Sg = S128[ei]
                nsub = (Se + 127) // 128
                w16 = Sg // 16
                # ---- token lists ----
                cwG = mb.tile([16, ntok // 16], f32, tag="cwG")
                nc.scalar.dma_start(
                    cwG[:], cbG[ei, :].rearrange("(f p) -> p f", p=16))
                cwT = mb.tile([16, ntok // 16], f32, tag="cwT")
                nc.scalar.dma_start(
                    cwT[:], cbT[ei, :].rearrange("(f p) -> p f", p=16))

                # junk-tail mask: slots >= count are diverted/neutralized
                msk = mb.tile([16, w16], f32, tag="msk")
                nc.vector.tensor_tensor(
                    out=msk[:], in0=slotio[:, :w16],
                    in1=cnt16[:, ei:ei + 1].to_broadcast([16, w16]),
                    op=ALU.is_lt)

                tkT = mb.tile([16, w16], f32, tag="tkT")
                nc.vector.memset(tkT[:], -1.0)
                nfdT = mb.tile([1, 1], dt.uint32, tag="nfdT")
                nc.gpsimd.sparse_gather(tkT[:, :Se // 16], cwT[:],
                                        num_found=nfdT[:])
                tkG = mb.tile([16, w16], f32, tag="tkG")
                nc.vector.memset(tkG[:], -1.0)
                nfdG = mb.tile([1, 1], dt.uint32, tag="nfdG")
                nc.gpsimd.sparse_gather(tkG[:, :Se // 16], cwG[:],
                                        num_found=nfdG[:])

                # x-row list: token ids, pads/junk clamped into range
                xff = mb.tile([16, w16], f32, tag="xff")
                nc.vector.tensor_scalar(xff[:], tkT[:], 1.0, float(ntok),
                                        op0=ALU.max, op1=ALU.min)
                nc.vector.tensor_scalar_add(xff[:], xff[:], -1.0)
                # scatter list: same tokens, junk tail -> trash row
                stf = mb.tile([16, Se // 16], f32, tag="stf")
                nc.vector.scalar_tensor_tensor(
                    out=stf[:], in0=xff[:, :Se // 16], scalar=float(-ntok),
                    in1=msk[:, :Se // 16], op0=ALU.add, op1=ALU.mult)
                nc.vector.tensor_scalar_add(stf[:], stf[:], float(ntok))
                # gate list: rows 1+2t+r of ginter, pads -> zero row
                gvf = mb.tile([16, w16], f32, tag="gvf")
                nc.vector.tensor_scalar(gvf[:], tkG[:], 0.0, float(2 * ntok),
                                        op0=ALU.max, op1=ALU.min)

                # single replicated index tile: [X | T | G]
                trip = mb.tile([P, 3 * w16], dt.int16, tag="trip")
                nc.vector.tensor_copy(trip[0:16, 0:w16], xff[:])
                nc.vector.tensor_copy(trip[0:16, w16:w16 + Se // 16], stf[:])
                nc.vector.tensor_copy(trip[0:16, 2 * w16:3 * w16], gvf[:])
                for sz in (16, 32, 64):
                    nc.scalar.dma_start(trip[sz:2 * sz, :], trip[0:sz, :])
                t16X = trip[:, 0:w16]
                t16T = trip[:, w16:w16 + Se // 16]
                t16G = trip[:, 2 * w16:3 * w16]

                # ---- dispatch gathers ----
                xg = xgp.tile([P, CK, Sg], bf16, tag="xg")
                nc.gpsimd.dma_gather(xg[:], xtm, t16X, Sg, Sg, C,
                                     transpose=True)
                ggt = xgp.tile([P, Sg // 128, GW64], f32, tag="gg")
                nc.gpsimd.dma_gather(ggt[:], gint, t16G, Sg, Sg, GW64)

                b1e = mb.tile([P, HK], f32, tag="b1e")
                nc.scalar.dma_start(
                    b1e[:].rearrange("p (k o) -> p k o", o=1),
                    b1[ei].rearrange("(k p) one -> p k one", p=P))

                # ---- w2 resident for this expert ----
                w2t = []
                for hk in range(HK):
                    wt = w2p.tile([P, C], bf16, tag=f"w2_{hk}")
                    nc.sync.dma_start(
                        wt[:], w2[ei, hk * P:(hk + 1) * P, :])
                    w2t.append(wt)

                yst = ystp.tile([P, nsub, C], bf16, tag="yst")
                if level < 3:
                    continue
                for (woff, W) in _tiles(Se):
                    # layer 1: out [h, slots]
                    hs = []
                    for hk in range(HK):
                        wrow = w1p.tile([P, CK * P], bf16, tag="w1r")
                        nc.sync.dma_start(wrow[:], w1[ei, hk])
                        ps = p1.tile([P, W], f32, space="PSUM", tag="ps1")
                        for k in range(CK):
                            nc.tensor.matmul(
                                ps[:], lhsT=wrow[:, k * P:(k + 1) * P],
                                rhs=xg[:, k, woff:woff + W],
                                start=(k == 0), stop=(k == CK - 1))
                        ht = hp.tile([P, W], bf16, tag=f"h{hk}")
                        nc.scalar.activation(ht[:], ps[:], AF.Relu,
                                             bias=b1e[:, hk:hk + 1])
                        hs.append(ht)
                    if level < 4:
                        continue
                    # layer 2: h stationary -> out token-major [slots, C]
                    for sub in range((W + 127) // 128):
                        lo = sub * 128
                        wsub = min(128, W - lo)
                        gsub = (woff + lo) // 128
                        psA = p2.tile([P, 512], f32, space="PSUM", tag="ps2a")
                        psB = p2.tile([P, 512], f32, space="PSUM", tag="ps2b")
                        for hk in range(HK):
                            nc.tensor.matmul(
                                psA[0:wsub, :],
                                lhsT=hs[hk][:, lo:lo + wsub],
                                rhs=w2t[hk][:, 0:512],
                                start=(hk == 0), stop=(hk == HK - 1))
                            nc.tensor.matmul(
                                psB[0:wsub, :],
                                lhsT=hs[hk][:, lo:lo + wsub],
                                rhs=w2t[hk][:, 512:1024],
                                start=(hk == 0), stop=(hk == HK - 1))
                        nc.scalar.activation(yst[0:wsub, gsub, 0:512],
                                             psA[0:wsub, :], AF.Copy,
                                             scale=ggt[0:wsub, gsub, 0:1])
                        nc.scalar.activation(yst[0:wsub, gsub, 512:1024],
                                             psB[0:wsub, :], AF.Copy,
                                             scale=ggt[0:wsub, gsub, 0:1])
                if level < 5:
                    continue
                # ---- combine: scatter-add token rows into yT ----
                nc.gpsimd.dma_scatter_add(yT, yst[:], t16T, Se, Se, C,
                                          queue_num=1)

        # software pipeline: stage A of rep+1 is emitted before stage B of
        # rep so the scheduler overlaps the routing chain with PE work
        actx = stage_a(0)
        for rep in range(repeat):
            nxt = stage_a(rep + 1) if rep + 1 < repeat else None
            if level >= 2 and actx is not None:
                stage_b(rep, actx)
            actx = nxt

    return nc


# ---------------- host side ----------------

def _host_caps(xf, gate_w, gate_b, ntok=NTOK, margin=16):
    """Slot capacities per expert from a host replica of the routing."""
    logits = xf.astype(np.float32) @ gate_w.astype(np.float32) + gate_b
    order = np.argpartition(-logits, TOPK - 1, axis=1)[:, :TOPK]
    ncore = xf.shape[0] // ntok
    counts = np.zeros((ncore, E), np.int64)
    for cc in range(ncore):
        sl = order[cc * ntok:(cc + 1) * ntok]
        counts[cc] = np.bincount(sl.ravel(), minlength=E)
    maxc = counts.max(axis=0)
    S = ((maxc + margin + 15) // 16) * 16
    assert S.max() <= 512, f"capacity overflow: {S}"
    return S.astype(np.int64)


def kernel(x, gate_w, gate_b, w1, b1, w2, b2):
    from concourse.bass_utils import run_bass_kernel_spmd
    import ml_dtypes

    x = np.asarray(x, np.float32)
    gate_w = np.asarray(gate_w, np.float32)
    gate_b = np.asarray(gate_b, np.float32)
    w1 = np.asarray(w1, np.float32)
    b1 = np.asarray(b1, np.float32)
    w2 = np.asarray(w2, np.float32)
    b2 = np.asarray(b2, np.float32)

    # w1 in lhsT-chunk layout: [E, HK, P(c in chunk), CK*P(h)]
    w1r = np.ascontiguousarray(
        (w1.reshape(E, CK, P, HK, P).transpose(0, 3, 2, 1, 4)
         .reshape(E, HK, P, C)).astype(ml_dtypes.bfloat16))
    w2b = np.ascontiguousarray(w2.astype(ml_dtypes.bfloat16))

    b, t, c = x.shape
    xf = x.reshape(b * t, c)
    S = _host_caps(xf, gate_w, gate_b)
    nc = build_program(S)

    shared = {
        "gw": gate_w,
        "gb": gate_b.reshape(E, 1).copy(),
        "w1": w1r,
        "b1": b1.reshape(E, H, 1).copy(),
        "w2": w2b,
        "b2e": b2,
        "id8": np.eye(E, dtype=np.float32),
        "id128": np.eye(P, dtype=np.float32),
    }
    in_maps = []
    for cc in range(NCORE):
        sl = xf[cc * NTOK:(cc + 1) * NTOK]
        m = dict(shared)
        m["xT"] = np.ascontiguousarray(sl.T)
        m["xtm"] = np.ascontiguousarray(sl.astype(ml_dtypes.bfloat16))
        in_maps.append(m)

    global LAST_BUILD, LAST_S
    LAST_BUILD = (nc, in_maps)
    LAST_S = S
    res = run_bass_kernel_spmd(nc, in_maps, core_ids=list(range(NCORE)))
    outs = [np.asarray(r["yT"][:NTOK]).astype(np.float32)
            for r in res.results]
    y = np.concatenate(outs, axis=0).reshape(b, t, c)
    return y
